# revision 1
# baseline (speedup 1.0000x reference)
"""RGB->hue + 1x1 conv (scalar scale+bias) Trainium2 Bass kernel.

Problem: x [32,3,512,512] f32 -> out [32,1,512,512] f32
  hue6 selected per argmax(r,g,b) branch:
    r max: (g-b)/delta  (mod 6)
    g max: (b-r)/delta + 2
    b max: (r-g)/delta + 4
  out = hue6 * (W/6) + b

Sharding: pure data parallel, 4 images per core on 8 cores.

Per-pixel engine split (per core: 4 images x 262144 px):
  GpSimd : 3 channel diffs (tensor_sub)
  ScalarE: 3 x Abs (for delta = max |diff|), 2 x final affine (q, q+6*w6)
  VectorE: max tree (2), reciprocal, 3 masks, 2 fused stt numerators,
           3 predicated copies, 1 multiply
The mod-6 wrap is handled by computing both final affines on ACT and
predicated-copying where the pre-division numerator is negative.
"""

import numpy as np

_EXE_CACHE: dict = {}

# Layout constants (hardcoded for x [32,3,512,512] f32, 8 cores)
N_CORES = 8
IMGS_PER_CORE = 4
P = 128              # SBUF partitions
PLANE = 512 * 512    # elements per channel plane
FREE = PLANE // P    # 2048 free-dim elements per plane
FD = 1024            # chunk free-dim size
CHUNKS = FREE // FD  # chunks per image plane


def _build(nc_mod, w6: float, bias: float):
    """Trace the Bass kernel with W/6 and bias baked as immediates."""
    import concourse.bacc as bacc
    import concourse.bass as bass
    import concourse.tile as tile
    from concourse import mybir

    F32 = mybir.dt.float32
    Alu = mybir.AluOpType
    Act = mybir.ActivationFunctionType
    ts = bass.ts

    nc = bacc.Bacc("TRN2", target_bir_lowering=False, debug=False)

    # Register 0.5 as a const AP (used as exact-threshold Relu bias)
    t05 = nc.alloc_sbuf_tensor("const-float32-0.5", [128, 1], F32)
    nc.gpsimd.memset(t05.ap(), 0.5)
    nc.const_aps.aps[(F32, 0.5)] = t05.ap()
    nc.all_engine_barrier()

    x_t = nc.dram_tensor("x", [IMGS_PER_CORE * 3, P, FREE], F32, kind="ExternalInput")
    o_t = nc.dram_tensor("out", [IMGS_PER_CORE, P, FREE], F32, kind="ExternalOutput")

    with tile.TileContext(nc, pool_alloc_mode="queue") as tc:
        with (
            tc.tile_pool(name="io", bufs=3) as io,
            tc.tile_pool(name="tmp", bufs=2) as tmp,
        ):
            for img in range(IMGS_PER_CORE):
                for h in range(CHUNKS):
                    r = io.tile([P, FD], F32, tag="r")
                    g = io.tile([P, FD], F32, tag="g")
                    b = io.tile([P, FD], F32, tag="b")
                    nc.sync.dma_start(r[:], x_t[img * 3 + 0, :, ts(h, FD)])
                    nc.sync.dma_start(g[:], x_t[img * 3 + 1, :, ts(h, FD)])
                    nc.sync.dma_start(b[:], x_t[img * 3 + 2, :, ts(h, FD)])

                    # Channel differences on GpSimd
                    drg = tmp.tile([P, FD], F32, tag="drg")
                    dbr = tmp.tile([P, FD], F32, tag="dbr")
                    dgb = tmp.tile([P, FD], F32, tag="dgb")
                    nc.gpsimd.tensor_sub(drg[:], r[:], g[:])
                    nc.gpsimd.tensor_sub(dbr[:], b[:], r[:])
                    nc.gpsimd.tensor_sub(dgb[:], g[:], b[:])

                    # |diffs| on ScalarE; delta = max(|drg|,|dbr|,|dgb|, eps)
                    a1 = tmp.tile([P, FD], F32, tag="a1")
                    a2 = tmp.tile([P, FD], F32, tag="a2")
                    a3 = tmp.tile([P, FD], F32, tag="a3")
                    nc.scalar.activation(a1[:], drg[:], Act.Abs)
                    nc.scalar.activation(a2[:], dbr[:], Act.Abs)
                    nc.scalar.activation(a3[:], dgb[:], Act.Abs)
                    # a1 <- max(a1, a2); a2 <- delta = max(a1, 1e-20, a3)
                    nc.vector.tensor_tensor(a1[:], a1[:], a2[:], op=Alu.max)
                    nc.vector.scalar_tensor_tensor(
                        a2[:], a1[:], 1e-20, a3[:], op0=Alu.max, op1=Alu.max
                    )
                    # a3 <- 1/delta
                    nc.vector.reciprocal(a3[:], a2[:])

                    # Branch masks (uint8):
                    #   c_m = (dgb>=0) exactly, via floor(Relu(100*dgb+1)) on ACT
                    #   a_m = (min(-dbr, drg) >= 0)  i.e. (r>=g) & (b<=r)
                    U8 = mybir.dt.uint8
                    c_m = tmp.tile([P, FD], U8, tag="c_m")
                    sa8 = tmp.tile([P, FD], U8, tag="sa8")
                    sb8 = tmp.tile([P, FD], U8, tag="sb8")
                    a_m = tmp.tile([P, FD], U8, tag="a_m")
                    nc.scalar.activation(
                        c_m[:], dgb[:], Act.Relu, bias=0.5, scale=100.0
                    )
                    nc.scalar.activation(
                        sa8[:], drg[:], Act.Relu, bias=0.5, scale=100.0
                    )
                    nc.scalar.activation(
                        sb8[:], dbr[:], Act.Relu, bias=0.5, scale=-100.0
                    )
                    nc.vector.scalar_tensor_tensor(
                        a_m[:], sa8[:], 0, sb8[:], op0=Alu.is_gt, op1=Alu.logical_and
                    )

                    # Pre-division numerators:
                    #   Nb = (r-g) + 4*delta   (b-max branch, base)
                    #   Ncand = (b-r) + 2*delta (g-max branch)
                    #   r-max branch numerator = dgb
                    Ncand = tmp.tile([P, FD], F32, tag="Ncand")
                    N = tmp.tile([P, FD], F32, tag="N")
                    nc.vector.scalar_tensor_tensor(
                        Ncand[:], a2[:], 2.0, dbr[:], op0=Alu.mult, op1=Alu.add
                    )
                    nc.vector.scalar_tensor_tensor(
                        N[:], a2[:], 4.0, drg[:], op0=Alu.mult, op1=Alu.add
                    )
                    nc.vector.copy_predicated(N[:], c_m[:], Ncand[:])
                    nc.vector.copy_predicated(N[:], a_m[:], dgb[:])

                    # neg mask (hue6 < 0 <=> N < 0) before N*recip
                    neg = tmp.tile([P, FD], U8, tag="neg")
                    nc.gpsimd.tensor_scalar(
                        out=neg[:], in0=N[:], scalar1=0.0, scalar2=None, op0=Alu.is_lt
                    )
                    # hue6 = N * (1/delta)  (in-place into N, on Pool)
                    nc.gpsimd.tensor_mul(N[:], N[:], a3[:])

                    # Final affine on ACT: q = hue6*w6 + bias ; qp adds 6*w6 (mod wrap)
                    q = tmp.tile([P, FD], F32, tag="q")
                    qp = tmp.tile([P, FD], F32, tag="qp")
                    nc.scalar.activation(q[:], N[:], Act.Copy, bias=bias, scale=w6)
                    nc.scalar.activation(
                        qp[:], N[:], Act.Copy, bias=bias + 6.0 * w6, scale=w6
                    )
                    nc.vector.copy_predicated(q[:], neg[:], qp[:])

                    nc.sync.dma_start(o_t[img, :, ts(h, FD)], q[:])

    nc.compile()
    return nc


def _get_nc(w6: float, bias: float):
    key = (w6, bias, FD)
    if key not in _EXE_CACHE:
        _EXE_CACHE[key] = _build(None, w6, bias)
    return _EXE_CACHE[key]


def _run(x, W, b, trace=False, tmpdir=None):
    from concourse.bass_utils import run_bass_kernel_spmd

    x = np.ascontiguousarray(np.asarray(x, dtype=np.float32))
    Wv = float(np.asarray(W).reshape(-1)[0])
    bv = float(np.asarray(b).reshape(-1)[0])
    w6 = Wv / 6.0

    nc = _get_nc(w6, bv)

    shards = x.reshape(N_CORES, IMGS_PER_CORE * 3, P, FREE)
    in_maps = [{"x": shards[i]} for i in range(N_CORES)]
    res = run_bass_kernel_spmd(
        nc, in_maps, list(range(N_CORES)), trace=trace, tmpdir=tmpdir
    )
    out = np.stack([res.results[i]["out"] for i in range(N_CORES)], axis=0)
    out = out.reshape(32, 1, 512, 512)
    return out, res


def kernel(x, W, b):
    out, _ = _run(x, W, b, trace=False)
    return out



# revision 2
# speedup vs baseline: 1.1009x; 1.1009x over previous
"""RGB->hue + 1x1 conv (scalar scale+bias) Trainium2 Bass kernel, v3.

Problem: x [32,3,512,512] f32 -> out [32,1,512,512] f32
  out = hue(x) * W + b, hue in [0,1).

Sharding: pure data parallel, 4 images per core on 8 cores.

v3 design: nearly the whole per-pixel computation is fused into four
custom DVE ops (single-pass uop chains, registered into dve_ops.OPS),
so each [128,4096] chunk (2 images) needs only 6 DVE + 1 Pool + 1 Act
instructions:

  d1 = r-g            [Pool tt, f32 in -> f16]
  d3 = r-b            [DVE tt, f32 in -> f16]
  delta = max(|d1|,|d3|,|d3-d1|)                  [HUE_DELTA_ANT, 5 uops]
  rcp = 1/delta                                   [Act Reciprocal table]
  P = r? d2 : (g? -d3 : d1)                       [HUE_PSEL_ANT, 8 uops]
  w = (c2-c1) - 2*c1*c2   (coeff = 2w+4)         [HUE_W_ANT, 8 uops]
  h0 = P * rcp                                    [DVE tt]
  q = w6*h0 + 2*w6*w + (4*w6 + bias)              [HUE_FIN_ANT, f32 out]

where d2 = d3-d1 = g-b, c1 = (d2>=0), c2 = (min(d1,d3)>=0) = r-is-max,
and the mod-6 wrap is folded into the w encoding (r-branch with d2<0
gets coeff 6). Diffs are computed from f32 inputs so their f16 error is
relative to the diff itself; hue error stays ~1e-3, far under the 2e-2
gate.
"""

import re

import numpy as np

_EXE_CACHE: dict = {}

N_CORES = 8
IMGS_PER_CORE = 4
P = 128
PLANE_F = 2048
IMGS_PER_CHUNK = 2
FD = PLANE_F * IMGS_PER_CHUNK   # 4096
CHUNKS = IMGS_PER_CORE // IMGS_PER_CHUNK


# ---- custom fused DVE ops -------------------------------------------------

_REGISTERED: dict = {}


def _register_fused_ops():
    """Define + register the fused hue ops in dve_ops.OPS (idempotent)."""
    if _REGISTERED:
        return _REGISTERED

    from concourse import dve_ops as D
    from concourse.dve_ops import DveOp
    from concourse.dve_spec import (
        Bin,
        C0,
        C1,
        C2,
        Spec,
        Src0,
        Src1,
        Zero,
        maxx,
        minn,
        select,
    )
    from concourse.dve_uop import AluOp

    def absd(a, b):
        return Bin(AluOp.ABSOLUTE_DIFF, a, b)

    d2 = Src1 - Src0  # Src0 = d1, Src1 = d3; d2 = g-b
    c2 = minn(Src0, Src1) >= Zero
    c1 = d2 >= Zero
    p = c1 * c2

    specs = {
        "HUE_DELTA_ANT": Spec(
            body=maxx(maxx(absd(Src0, Zero), absd(Src1, Zero)), absd(Src1, Src0)),
            reference=lambda in0, in1, s0, s1, imm2: np.maximum(
                np.maximum(np.abs(in0), np.abs(in1)), np.abs(in1 - in0)
            ),
        ),
        "HUE_PSEL_ANT": Spec(
            body=select(c2, d2, select(c1, Zero - Src1, Src0)),
            reference=lambda in0, in1, s0, s1, imm2: np.where(
                np.minimum(in0, in1) >= 0,
                in1 - in0,
                np.where((in1 - in0) >= 0, -in1, in0),
            ),
        ),
        "HUE_W_ANT": Spec(
            body=(c2 - c1) - (p + p),
            reference=lambda in0, in1, s0, s1, imm2: (
                (np.minimum(in0, in1) >= 0).astype(np.float32)
                - ((in1 - in0) >= 0).astype(np.float32)
                - 2.0
                * ((in1 - in0) >= 0).astype(np.float32)
                * (np.minimum(in0, in1) >= 0).astype(np.float32)
            ),
        ),
        "HUE_FIN_ANT": Spec(
            body=Src0 * C0 + Src1 * C1 + C2,
            reference=lambda in0, in1, s0, s1, imm2: in0 * s0 + in1 * s1 + imm2,
        ),
    }

    for name, spec in specs.items():
        existing = [op for op in D.OPS if op.name == name]
        if existing:
            _REGISTERED[name] = existing[0]
            continue
        op = DveOp(name, spec, subdim=False, uops_sha={})
        D.OPS.append(op)
        D.CUSTOM_DVE_SPECS[name] = spec
        D._SUB_OPCODE_FOR_NAME[name] = D._CUSTOM_DVE_ROW_BASE + len(D.OPS) - 1
        assert D._SUB_OPCODE_FOR_NAME[name] < 0x20
        try:
            op.compile("v3")
        except ValueError as e:
            m = re.search(r'="([0-9a-f]+)"', str(e))
            if not m:
                raise
            op.uops_sha["v3"] = m.group(1)
            op.compile("v3")
        _REGISTERED[name] = op
    return _REGISTERED


def _act_reciprocal(nc, out_ap, in_ap):
    """Emit Act-table Reciprocal directly (bass wrapper refuses it)."""
    from concourse import mybir

    S = nc.scalar
    imm = lambda v: mybir.ImmediateValue(dtype=mybir.dt.float32, value=v)
    return S.add_instruction(
        mybir.InstActivation(
            name=nc.get_next_instruction_name(),
            func=mybir.ActivationFunctionType.Reciprocal,
            ins=[S.lower_ap(in_ap), imm(0.0), imm(1.0), imm(0.0)],
            outs=[S.lower_ap(out_ap)],
        )
    )


def _build(w6: float, bias: float):
    import concourse.bacc as bacc
    import concourse.tile as tile
    from concourse import mybir

    ops = _register_fused_ops()

    F32 = mybir.dt.float32
    F16 = mybir.dt.float16
    Alu = mybir.AluOpType

    nc = bacc.Bacc("TRN2", target_bir_lowering=False, debug=False)

    t05 = nc.alloc_sbuf_tensor("const-float32-0.5", [128, 1], F32)
    nc.gpsimd.memset(t05.ap(), 0.5)
    nc.const_aps.aps[(F32, 0.5)] = t05.ap()
    nc.all_engine_barrier()

    x_t = nc.dram_tensor("x", [IMGS_PER_CORE * 3, P, PLANE_F], F32, kind="ExternalInput")
    o_t = nc.dram_tensor("out", [IMGS_PER_CORE, P, PLANE_F], F32, kind="ExternalOutput")

    V = nc.vector
    G = nc.gpsimd
    sl = lambda j: slice(j * PLANE_F, (j + 1) * PLANE_F)

    with tile.TileContext(nc, pool_alloc_mode="queue") as tc:
        with (
            tc.tile_pool(name="io", bufs=2) as io,
            tc.tile_pool(name="wk", bufs=1) as wk,
        ):
            for ch in range(CHUNKS):
                i0 = ch * IMGS_PER_CHUNK
                r = io.tile([P, FD], F32, tag="r")
                g = io.tile([P, FD], F32, tag="g")
                b = io.tile([P, FD], F32, tag="b")
                for j in range(IMGS_PER_CHUNK):
                    nc.sync.dma_start(r[:, sl(j)], x_t[(i0 + j) * 3 + 0, :, :])
                    nc.sync.dma_start(g[:, sl(j)], x_t[(i0 + j) * 3 + 1, :, :])
                    nc.sync.dma_start(b[:, sl(j)], x_t[(i0 + j) * 3 + 2, :, :])

                d1 = wk.tile([P, FD], F16, tag="d1")
                d3 = wk.tile([P, FD], F16, tag="d3")
                G.tensor_tensor(d1[:], r[:], g[:], op=Alu.subtract)
                V.tensor_tensor(d3[:], r[:], b[:], op=Alu.subtract)

                delta = wk.tile([P, FD], F16, tag="delta")
                V._custom_dve(ops["HUE_DELTA_ANT"], out=delta[:], in0=d1[:], in1=d3[:])

                rcp = wk.tile([P, FD], F16, tag="rcp")
                _act_reciprocal(nc, rcp[:], delta[:])

                Pt = wk.tile([P, FD], F16, tag="P")
                wt = wk.tile([P, FD], F16, tag="w")
                V._custom_dve(ops["HUE_PSEL_ANT"], out=Pt[:], in0=d1[:], in1=d3[:])
                V._custom_dve(ops["HUE_W_ANT"], out=wt[:], in0=d1[:], in1=d3[:])

                # h0 = P * rcp (in place into P)
                V.tensor_tensor(Pt[:], Pt[:], rcp[:], op=Alu.mult)

                q = wk.tile([P, FD], F32, tag="q")
                V._custom_dve(
                    ops["HUE_FIN_ANT"],
                    out=q[:],
                    in0=Pt[:],
                    in1=wt[:],
                    s0=w6,
                    s1=2.0 * w6,
                    imm2=4.0 * w6 + bias,
                )

                for j in range(IMGS_PER_CHUNK):
                    nc.sync.dma_start(o_t[i0 + j, :, :], q[:, sl(j)])

    nc.compile()
    return nc


def _get_nc(w6: float, bias: float):
    key = (w6, bias)
    if key not in _EXE_CACHE:
        _EXE_CACHE[key] = _build(w6, bias)
    return _EXE_CACHE[key]


def _run(x, W, b, trace=False, tmpdir=None):
    from concourse.bass_utils import run_bass_kernel_spmd

    x = np.ascontiguousarray(np.asarray(x, dtype=np.float32))
    Wv = float(np.asarray(W).reshape(-1)[0])
    bv = float(np.asarray(b).reshape(-1)[0])
    w6 = Wv / 6.0

    nc = _get_nc(w6, bv)

    shards = x.reshape(N_CORES, IMGS_PER_CORE * 3, P, PLANE_F)
    in_maps = [{"x": shards[i]} for i in range(N_CORES)]
    res = run_bass_kernel_spmd(
        nc, in_maps, list(range(N_CORES)), trace=trace, tmpdir=tmpdir
    )
    out = np.stack([res.results[i]["out"] for i in range(N_CORES)], axis=0)
    out = out.reshape(32, 1, 512, 512)
    return out, res


def kernel(x, W, b):
    out, _ = _run(x, W, b, trace=False)
    return out


# revision 3
# speedup vs baseline: 1.1327x; 1.0289x over previous
"""RGB->hue + 1x1 conv (scalar scale+bias) Trainium2 Bass kernel, v6.

Problem: x [32,3,512,512] f32 -> out [32,1,512,512] f32
  out = hue(x) * W + b, hue in [0,1).

Sharding: pure data parallel, 4 images per core on 8 cores.

Per [128,4096] chunk (2 images, fp16 intermediates, double-buffered):
  d1 = r-g [Pool, f32->f16]      d3 = r-b [DVE, f32->f16]   (parallel)
  delta = max(|d1|,|d3|,|d3-d1|, 2^-15)      [custom DVE, 1 op]
  rcp = 1/delta                              [Act Reciprocal table]
  P = r? d2 : (g? -d3 : d1)                  [custom DVE, 1 op]
  w = (c2-c1) - 2*c1*c2   (coeff = 2w+4)     [custom DVE, in-place on d1]
  h0 = P * rcp                               [DVE tt, in-place on P]
  q = w6*h0 + 2*w6*w + (4*w6 + bias)         [custom DVE, f32 out]
Input loads are striped across DMA queues (r,b first so d3/d1 start
early); stores are issued from the Act engine's DGE.
"""

import re

import numpy as np

_EXE_CACHE: dict = {}

N_CORES = 8
IMGS_PER_CORE = 4
P = 128
PLANE_F = 2048
IMGS_PER_CHUNK = 2
FD = PLANE_F * IMGS_PER_CHUNK   # 4096
CHUNKS = IMGS_PER_CORE // IMGS_PER_CHUNK
NS = 4                          # DMA strips per plane


_REGISTERED: dict = {}


def _register_fused_ops():
    if _REGISTERED:
        return _REGISTERED

    from concourse import dve_ops as D
    from concourse.dve_ops import DveOp
    from concourse.dve_spec import (
        Bin,
        C0,
        C1,
        C2,
        Spec,
        Src0,
        Src1,
        Zero,
        maxx,
        minn,
        select,
    )
    from concourse.dve_uop import AluOp

    def absd(a, b):
        return Bin(AluOp.ABSOLUTE_DIFF, a, b)

    d2 = Src1 - Src0
    c2 = minn(Src0, Src1) >= Zero
    c1 = d2 >= Zero
    p = c1 * c2

    specs = {
        "HUE_DELTA_ANT": Spec(
            body=maxx(
                maxx(maxx(absd(Src0, Zero), absd(Src1, Zero)), absd(Src1, Src0)),
                C0,
            ),
            reference=lambda in0, in1, s0, s1, imm2: np.maximum(
                np.maximum(np.maximum(np.abs(in0), np.abs(in1)), np.abs(in1 - in0)),
                s0,
            ),
        ),
        "HUE_PSEL_ANT": Spec(
            body=select(c2, d2, select(c1, Zero - Src1, Src0)),
            reference=lambda in0, in1, s0, s1, imm2: np.where(
                np.minimum(in0, in1) >= 0,
                in1 - in0,
                np.where((in1 - in0) >= 0, -in1, in0),
            ),
        ),
        "HUE_W_ANT": Spec(
            body=(c2 - c1) - (p + p),
            reference=lambda in0, in1, s0, s1, imm2: (
                (np.minimum(in0, in1) >= 0).astype(np.float32)
                - ((in1 - in0) >= 0).astype(np.float32)
                - 2.0
                * ((in1 - in0) >= 0).astype(np.float32)
                * (np.minimum(in0, in1) >= 0).astype(np.float32)
            ),
        ),
        "HUE_FIN_ANT": Spec(
            body=Src0 * C0 + Src1 * C1 + C2,
            reference=lambda in0, in1, s0, s1, imm2: in0 * s0 + in1 * s1 + imm2,
        ),
    }

    for name, spec in specs.items():
        existing = [op for op in D.OPS if op.name == name]
        if existing:
            _REGISTERED[name] = existing[0]
            continue
        op = DveOp(name, spec, subdim=False, uops_sha={})
        D.OPS.append(op)
        D.CUSTOM_DVE_SPECS[name] = spec
        D._SUB_OPCODE_FOR_NAME[name] = D._CUSTOM_DVE_ROW_BASE + len(D.OPS) - 1
        assert D._SUB_OPCODE_FOR_NAME[name] < 0x20
        try:
            op.compile("v3")
        except ValueError as e:
            m = re.search(r'="([0-9a-f]+)"', str(e))
            if not m:
                raise
            op.uops_sha["v3"] = m.group(1)
            op.compile("v3")
        _REGISTERED[name] = op
    return _REGISTERED


def _act_reciprocal(nc, out_ap, in_ap):
    from concourse import mybir

    S = nc.scalar
    imm = lambda v: mybir.ImmediateValue(dtype=mybir.dt.float32, value=v)
    return S.add_instruction(
        mybir.InstActivation(
            name=nc.get_next_instruction_name(),
            func=mybir.ActivationFunctionType.Reciprocal,
            ins=[S.lower_ap(in_ap), imm(0.0), imm(1.0), imm(0.0)],
            outs=[S.lower_ap(out_ap)],
        )
    )


def _build(w6: float, bias: float):
    import concourse.bacc as bacc
    import concourse.tile as tile
    from concourse import mybir

    ops = _register_fused_ops()

    F32 = mybir.dt.float32
    F16 = mybir.dt.float16
    Alu = mybir.AluOpType

    nc = bacc.Bacc("TRN2", target_bir_lowering=False, debug=False)

    x_t = nc.dram_tensor("x", [IMGS_PER_CORE * 3, P, PLANE_F], F32, kind="ExternalInput")
    o_t = nc.dram_tensor("out", [IMGS_PER_CORE, P, PLANE_F], F32, kind="ExternalOutput")

    V = nc.vector
    G = nc.gpsimd
    SW = PLANE_F // NS

    with tile.TileContext(nc, pool_alloc_mode="queue") as tc:
        with (
            tc.tile_pool(name="io", bufs=2) as io,
            tc.tile_pool(name="wk", bufs=2) as wk,
        ):
            for ch in range(CHUNKS):
                i0 = ch * IMGS_PER_CHUNK
                r = io.tile([P, FD], F32, tag="r")
                g = io.tile([P, FD], F32, tag="g")
                b = io.tile([P, FD], F32, tag="b")
                # r,b strips first so the subs can start ASAP
                for t, c in ((r, 0), (b, 2), (g, 1)):
                    for j in range(IMGS_PER_CHUNK):
                        for k in range(NS):
                            fs = slice(j * PLANE_F + k * SW, j * PLANE_F + (k + 1) * SW)
                            eng = nc.sync if (k % 2 == 0) else nc.scalar
                            eng.dma_start(
                                t[:, fs], x_t[(i0 + j) * 3 + c, :, k * SW:(k + 1) * SW]
                            )

                d1 = wk.tile([P, FD], F16, tag="d1")
                d3 = wk.tile([P, FD], F16, tag="d3")
                # per-image halves: start each as soon as its planes land.
                # d3 (r,b: loaded first) -> Pool; d1 (needs g) -> DVE.
                for j in range(IMGS_PER_CHUNK):
                    hs = slice(j * PLANE_F, (j + 1) * PLANE_F)
                    G.tensor_tensor(d3[:, hs], r[:, hs], b[:, hs], op=Alu.subtract)
                for j in range(IMGS_PER_CHUNK):
                    hs = slice(j * PLANE_F, (j + 1) * PLANE_F)
                    V.tensor_tensor(d1[:, hs], r[:, hs], g[:, hs], op=Alu.subtract)

                delta = wk.tile([P, FD], F16, tag="delta")
                V._custom_dve(
                    ops["HUE_DELTA_ANT"], out=delta[:], in0=d1[:], in1=d3[:],
                    s0=2.0 ** -15,
                )
                # rcp in place over delta (Act)
                _act_reciprocal(nc, delta[:], delta[:])
                rcp = delta

                Pt = wk.tile([P, FD], F16, tag="P")
                V._custom_dve(ops["HUE_PSEL_ANT"], out=Pt[:], in0=d1[:], in1=d3[:])
                # w in place over d1 (last reader of d1/d3)
                V._custom_dve(ops["HUE_W_ANT"], out=d1[:], in0=d1[:], in1=d3[:])
                wt = d1

                # h0 = P * rcp in place over P
                V.tensor_tensor(Pt[:], Pt[:], rcp[:], op=Alu.mult)

                q = wk.tile([P, FD], F32, tag="q")
                V._custom_dve(
                    ops["HUE_FIN_ANT"], out=q[:], in0=Pt[:], in1=wt[:],
                    s0=w6, s1=2.0 * w6, imm2=4.0 * w6 + bias,
                )

                NSS = 8
                SWS = PLANE_F // NSS
                for j in range(IMGS_PER_CHUNK):
                    for k in range(NSS):
                        fs = slice(j * PLANE_F + k * SWS, j * PLANE_F + (k + 1) * SWS)
                        eng = nc.scalar if (k % 2 == 0) else nc.sync
                        eng.dma_start(
                            o_t[i0 + j, :, k * SWS:(k + 1) * SWS], q[:, fs]
                        )

    nc.compile()
    return nc


def _get_nc(w6: float, bias: float):
    key = (w6, bias)
    if key not in _EXE_CACHE:
        _EXE_CACHE[key] = _build(w6, bias)
    return _EXE_CACHE[key]


def _run(x, W, b, trace=False, tmpdir=None):
    from concourse.bass_utils import run_bass_kernel_spmd

    x = np.ascontiguousarray(np.asarray(x, dtype=np.float32))
    Wv = float(np.asarray(W).reshape(-1)[0])
    bv = float(np.asarray(b).reshape(-1)[0])
    w6 = Wv / 6.0

    nc = _get_nc(w6, bv)

    shards = x.reshape(N_CORES, IMGS_PER_CORE * 3, P, PLANE_F)
    in_maps = [{"x": shards[i]} for i in range(N_CORES)]
    res = run_bass_kernel_spmd(
        nc, in_maps, list(range(N_CORES)), trace=trace, tmpdir=tmpdir
    )
    out = np.stack([res.results[i]["out"] for i in range(N_CORES)], axis=0)
    out = out.reshape(32, 1, 512, 512)
    return out, res


def kernel(x, W, b):
    out, _ = _run(x, W, b, trace=False)
    return out


# revision 4
# speedup vs baseline: 1.1385x; 1.0051x over previous
"""RGB->hue + 1x1 conv (scalar scale+bias) Trainium2 Bass kernel, v6.

Problem: x [32,3,512,512] f32 -> out [32,1,512,512] f32
  out = hue(x) * W + b, hue in [0,1).

Sharding: pure data parallel, 4 images per core on 8 cores.

Per [128,4096] chunk (2 images, fp16 intermediates, double-buffered):
  d1 = r-g [Pool, f32->f16]      d3 = r-b [DVE, f32->f16]   (parallel)
  delta = max(|d1|,|d3|,|d3-d1|, 2^-15)      [custom DVE, 1 op]
  rcp = 1/delta                              [Act Reciprocal table]
  P = r? d2 : (g? -d3 : d1)                  [custom DVE, 1 op]
  w = (c2-c1) - 2*c1*c2   (coeff = 2w+4)     [custom DVE, in-place on d1]
  h0 = P * rcp                               [DVE tt, in-place on P]
  q = w6*h0 + 2*w6*w + (4*w6 + bias)         [custom DVE, f32 out]
Input loads are striped across DMA queues (r,b first so d3/d1 start
early); stores are issued from the Act engine's DGE.
"""

import re

import numpy as np

_EXE_CACHE: dict = {}

N_CORES = 8
IMGS_PER_CORE = 4
P = 128
PLANE_F = 2048
IMGS_PER_CHUNK = 2
FD = PLANE_F * IMGS_PER_CHUNK   # 4096
CHUNKS = IMGS_PER_CORE // IMGS_PER_CHUNK
NS = 4                          # DMA strips per plane


_REGISTERED: dict = {}


def _register_fused_ops():
    if _REGISTERED:
        return _REGISTERED

    from concourse import dve_ops as D
    from concourse.dve_ops import DveOp
    from concourse.dve_spec import (
        Bin,
        C0,
        C1,
        C2,
        Spec,
        Src0,
        Src1,
        Zero,
        maxx,
        minn,
        select,
    )
    from concourse.dve_uop import AluOp

    def absd(a, b):
        return Bin(AluOp.ABSOLUTE_DIFF, a, b)

    d2 = Src1 - Src0
    c2 = minn(Src0, Src1) >= Zero
    c1 = d2 >= Zero
    p = c1 * c2

    specs = {
        "HUE_DELTA_ANT": Spec(
            body=maxx(
                maxx(maxx(absd(Src0, Zero), absd(Src1, Zero)), absd(Src1, Src0)),
                C0,
            ),
            reference=lambda in0, in1, s0, s1, imm2: np.maximum(
                np.maximum(np.maximum(np.abs(in0), np.abs(in1)), np.abs(in1 - in0)),
                s0,
            ),
        ),
        "HUE_PSEL_ANT": Spec(
            body=select(c2, d2, select(c1, Zero - Src1, Src0)),
            reference=lambda in0, in1, s0, s1, imm2: np.where(
                np.minimum(in0, in1) >= 0,
                in1 - in0,
                np.where((in1 - in0) >= 0, -in1, in0),
            ),
        ),
        "HUE_W_ANT": Spec(
            body=(c2 - c1) - (p + p),
            reference=lambda in0, in1, s0, s1, imm2: (
                (np.minimum(in0, in1) >= 0).astype(np.float32)
                - ((in1 - in0) >= 0).astype(np.float32)
                - 2.0
                * ((in1 - in0) >= 0).astype(np.float32)
                * (np.minimum(in0, in1) >= 0).astype(np.float32)
            ),
        ),
        "HUE_FIN_ANT": Spec(
            body=Src0 * C0 + Src1 * C1 + C2,
            reference=lambda in0, in1, s0, s1, imm2: in0 * s0 + in1 * s1 + imm2,
        ),
    }

    for name, spec in specs.items():
        existing = [op for op in D.OPS if op.name == name]
        if existing:
            _REGISTERED[name] = existing[0]
            continue
        op = DveOp(name, spec, subdim=False, uops_sha={})
        D.OPS.append(op)
        D.CUSTOM_DVE_SPECS[name] = spec
        D._SUB_OPCODE_FOR_NAME[name] = D._CUSTOM_DVE_ROW_BASE + len(D.OPS) - 1
        assert D._SUB_OPCODE_FOR_NAME[name] < 0x20
        try:
            op.compile("v3")
        except ValueError as e:
            m = re.search(r'="([0-9a-f]+)"', str(e))
            if not m:
                raise
            op.uops_sha["v3"] = m.group(1)
            op.compile("v3")
        _REGISTERED[name] = op
    return _REGISTERED


def _act_reciprocal(nc, out_ap, in_ap):
    from concourse import mybir

    S = nc.scalar
    imm = lambda v: mybir.ImmediateValue(dtype=mybir.dt.float32, value=v)
    return S.add_instruction(
        mybir.InstActivation(
            name=nc.get_next_instruction_name(),
            func=mybir.ActivationFunctionType.Reciprocal,
            ins=[S.lower_ap(in_ap), imm(0.0), imm(1.0), imm(0.0)],
            outs=[S.lower_ap(out_ap)],
        )
    )


def _build(w6: float, bias: float):
    import concourse.bacc as bacc
    import concourse.tile as tile
    from concourse import mybir

    ops = _register_fused_ops()

    F32 = mybir.dt.float32
    F16 = mybir.dt.float16
    Alu = mybir.AluOpType

    nc = bacc.Bacc("TRN2", target_bir_lowering=False, debug=False)

    x_t = nc.dram_tensor("x", [IMGS_PER_CORE * 3, P, PLANE_F], F32, kind="ExternalInput")
    o_t = nc.dram_tensor("out", [IMGS_PER_CORE, P, PLANE_F], F32, kind="ExternalOutput")

    V = nc.vector
    G = nc.gpsimd
    SW = PLANE_F // NS

    with tile.TileContext(nc, pool_alloc_mode="queue") as tc:
        with (
            tc.tile_pool(name="io", bufs=2) as io,
            tc.tile_pool(name="wk", bufs=2) as wk,
        ):
            for ch in range(CHUNKS):
                i0 = ch * IMGS_PER_CHUNK
                r = io.tile([P, FD], F32, tag="r")
                g = io.tile([P, FD], F32, tag="g")
                b = io.tile([P, FD], F32, tag="b")
                # r,b strips first so the subs can start ASAP
                for j in range(IMGS_PER_CHUNK):
                    for t, c in ((r, 0), (b, 2), (g, 1)):
                        for k in range(NS):
                            fs = slice(j * PLANE_F + k * SW, j * PLANE_F + (k + 1) * SW)
                            eng = nc.sync if (k % 2 == 0) else nc.scalar
                            eng.dma_start(
                                t[:, fs], x_t[(i0 + j) * 3 + c, :, k * SW:(k + 1) * SW]
                            )

                d1 = wk.tile([P, FD], F16, tag="d1")
                d3 = wk.tile([P, FD], F16, tag="d3")
                # per-image halves: start each as soon as its planes land.
                # d3 (r,b: loaded first) -> Pool; d1 (needs g) -> DVE.
                for j in range(IMGS_PER_CHUNK):
                    hs = slice(j * PLANE_F, (j + 1) * PLANE_F)
                    G.tensor_tensor(d3[:, hs], r[:, hs], b[:, hs], op=Alu.subtract)
                for j in range(IMGS_PER_CHUNK):
                    hs = slice(j * PLANE_F, (j + 1) * PLANE_F)
                    V.tensor_tensor(d1[:, hs], r[:, hs], g[:, hs], op=Alu.subtract)

                delta = wk.tile([P, FD], F16, tag="delta")
                V._custom_dve(
                    ops["HUE_DELTA_ANT"], out=delta[:], in0=d1[:], in1=d3[:],
                    s0=2.0 ** -15,
                )
                # rcp in place over delta (Act)
                _act_reciprocal(nc, delta[:], delta[:])
                rcp = delta

                Pt = wk.tile([P, FD], F16, tag="P")
                V._custom_dve(ops["HUE_PSEL_ANT"], out=Pt[:], in0=d1[:], in1=d3[:])
                # w in place over d1 (last reader of d1/d3)
                V._custom_dve(ops["HUE_W_ANT"], out=d1[:], in0=d1[:], in1=d3[:])
                wt = d1

                # h0 = P * rcp in place over P
                V.tensor_tensor(Pt[:], Pt[:], rcp[:], op=Alu.mult)

                q = wk.tile([P, FD], F32, tag="q")
                V._custom_dve(
                    ops["HUE_FIN_ANT"], out=q[:], in0=Pt[:], in1=wt[:],
                    s0=w6, s1=2.0 * w6, imm2=4.0 * w6 + bias,
                )

                NSS = 8
                SWS = PLANE_F // NSS
                for j in range(IMGS_PER_CHUNK):
                    for k in range(NSS):
                        fs = slice(j * PLANE_F + k * SWS, j * PLANE_F + (k + 1) * SWS)
                        eng = nc.scalar if (k % 2 == 0) else nc.sync
                        eng.dma_start(
                            o_t[i0 + j, :, k * SWS:(k + 1) * SWS], q[:, fs]
                        )

    nc.compile()
    return nc


def _get_nc(w6: float, bias: float):
    key = (w6, bias)
    if key not in _EXE_CACHE:
        _EXE_CACHE[key] = _build(w6, bias)
    return _EXE_CACHE[key]


def _run(x, W, b, trace=False, tmpdir=None):
    from concourse.bass_utils import run_bass_kernel_spmd

    x = np.ascontiguousarray(np.asarray(x, dtype=np.float32))
    Wv = float(np.asarray(W).reshape(-1)[0])
    bv = float(np.asarray(b).reshape(-1)[0])
    w6 = Wv / 6.0

    nc = _get_nc(w6, bv)

    shards = x.reshape(N_CORES, IMGS_PER_CORE * 3, P, PLANE_F)
    in_maps = [{"x": shards[i]} for i in range(N_CORES)]
    res = run_bass_kernel_spmd(
        nc, in_maps, list(range(N_CORES)), trace=trace, tmpdir=tmpdir
    )
    out = np.stack([res.results[i]["out"] for i in range(N_CORES)], axis=0)
    out = out.reshape(32, 1, 512, 512)
    return out, res


def kernel(x, W, b):
    out, _ = _run(x, W, b, trace=False)
    return out


# revision 5
# speedup vs baseline: 1.1828x; 1.0389x over previous
"""RGB->hue + 1x1 conv (scalar scale+bias) Trainium2 Bass kernel, v6.

Problem: x [32,3,512,512] f32 -> out [32,1,512,512] f32
  out = hue(x) * W + b, hue in [0,1).

Sharding: pure data parallel, 4 images per core on 8 cores.

Per [128,4096] chunk (2 images, fp16 intermediates, double-buffered):
  d1 = r-g [Pool, f32->f16]      d3 = r-b [DVE, f32->f16]   (parallel)
  delta = max(|d1|,|d3|,|d3-d1|, 2^-15)      [custom DVE, 1 op]
  rcp = 1/delta                              [Act Reciprocal table]
  P = r? d2 : (g? -d3 : d1)                  [custom DVE, 1 op]
  w = (c2-c1) - 2*c1*c2   (coeff = 2w+4)     [custom DVE, in-place on d1]
  h0 = P * rcp                               [DVE tt, in-place on P]
  q = w6*h0 + 2*w6*w + (4*w6 + bias)         [custom DVE, f32 out]
Input loads are striped across DMA queues (r,b first so d3/d1 start
early); stores are issued from the Act engine's DGE.
"""

import re

import numpy as np

_EXE_CACHE: dict = {}

N_CORES = 8
IMGS_PER_CORE = 4
P = 128
PLANE_F = 2048
IMGS_PER_CHUNK = 2
FD = PLANE_F * IMGS_PER_CHUNK   # 4096
CHUNKS = IMGS_PER_CORE // IMGS_PER_CHUNK
NS = 4                          # DMA strips per plane


_REGISTERED: dict = {}


def _register_fused_ops():
    if _REGISTERED:
        return _REGISTERED

    from concourse import dve_ops as D
    from concourse.dve_ops import DveOp
    from concourse.dve_spec import (
        Bin,
        C0,
        C1,
        C2,
        Spec,
        Src0,
        Src1,
        Zero,
        maxx,
        minn,
        select,
    )
    from concourse.dve_uop import AluOp

    def absd(a, b):
        return Bin(AluOp.ABSOLUTE_DIFF, a, b)

    d2 = Src1 - Src0
    c2 = minn(Src0, Src1) >= Zero
    c1 = d2 >= Zero
    p = c1 * c2

    specs = {
        "HUE_DELTA_ANT": Spec(
            body=maxx(
                maxx(maxx(absd(Src0, Zero), absd(Src1, Zero)), absd(Src1, Src0)),
                C0,
            ),
            reference=lambda in0, in1, s0, s1, imm2: np.maximum(
                np.maximum(np.maximum(np.abs(in0), np.abs(in1)), np.abs(in1 - in0)),
                s0,
            ),
        ),
        "HUE_PSEL_ANT": Spec(
            body=select(c2, d2, select(c1, Zero - Src1, Src0)),
            reference=lambda in0, in1, s0, s1, imm2: np.where(
                np.minimum(in0, in1) >= 0,
                in1 - in0,
                np.where((in1 - in0) >= 0, -in1, in0),
            ),
        ),
        "HUE_W_ANT": Spec(
            body=(c2 - c1) - (p + p),
            reference=lambda in0, in1, s0, s1, imm2: (
                (np.minimum(in0, in1) >= 0).astype(np.float32)
                - ((in1 - in0) >= 0).astype(np.float32)
                - 2.0
                * ((in1 - in0) >= 0).astype(np.float32)
                * (np.minimum(in0, in1) >= 0).astype(np.float32)
            ),
        ),
        "HUE_FIN_ANT": Spec(
            body=Src0 * C0 + Src1 * C1 + C2,
            reference=lambda in0, in1, s0, s1, imm2: in0 * s0 + in1 * s1 + imm2,
        ),
    }

    for name, spec in specs.items():
        existing = [op for op in D.OPS if op.name == name]
        if existing:
            _REGISTERED[name] = existing[0]
            continue
        op = DveOp(name, spec, subdim=False, uops_sha={})
        D.OPS.append(op)
        D.CUSTOM_DVE_SPECS[name] = spec
        D._SUB_OPCODE_FOR_NAME[name] = D._CUSTOM_DVE_ROW_BASE + len(D.OPS) - 1
        assert D._SUB_OPCODE_FOR_NAME[name] < 0x20
        try:
            op.compile("v3")
        except ValueError as e:
            m = re.search(r'="([0-9a-f]+)"', str(e))
            if not m:
                raise
            op.uops_sha["v3"] = m.group(1)
            op.compile("v3")
        _REGISTERED[name] = op
    return _REGISTERED


def _act_reciprocal(nc, out_ap, in_ap):
    from concourse import mybir

    S = nc.scalar
    imm = lambda v: mybir.ImmediateValue(dtype=mybir.dt.float32, value=v)
    return S.add_instruction(
        mybir.InstActivation(
            name=nc.get_next_instruction_name(),
            func=mybir.ActivationFunctionType.Reciprocal,
            ins=[S.lower_ap(in_ap), imm(0.0), imm(1.0), imm(0.0)],
            outs=[S.lower_ap(out_ap)],
        )
    )


def _build(w6: float, bias: float):
    import concourse.bacc as bacc
    import concourse.tile as tile
    from concourse import mybir

    ops = _register_fused_ops()

    F32 = mybir.dt.float32
    F16 = mybir.dt.float16
    Alu = mybir.AluOpType

    nc = bacc.Bacc("TRN2", target_bir_lowering=False, debug=False)

    x_t = nc.dram_tensor("x", [IMGS_PER_CORE * 3, P, PLANE_F], F32, kind="ExternalInput")
    o_t = nc.dram_tensor("out", [IMGS_PER_CORE, P, PLANE_F], F32, kind="ExternalOutput")

    V = nc.vector
    G = nc.gpsimd
    SW = PLANE_F // NS

    with tile.TileContext(nc, pool_alloc_mode="queue") as tc:
        with (
            tc.tile_pool(name="io", bufs=2) as io,
            tc.tile_pool(name="wk", bufs=2) as wk,
        ):
            for ch in range(CHUNKS):
                i0 = ch * IMGS_PER_CHUNK
                r = io.tile([P, FD], F32, tag="r")
                g = io.tile([P, FD], F32, tag="g")
                b = io.tile([P, FD], F32, tag="b")
                # r,b strips first so the subs can start ASAP
                for j in range(IMGS_PER_CHUNK):
                    for t, c in ((r, 0), (b, 2), (g, 1)):
                        for k in range(NS):
                            fs = slice(j * PLANE_F + k * SW, j * PLANE_F + (k + 1) * SW)
                            eng = nc.sync if (k % 2 == 0) else nc.scalar
                            eng.dma_start(
                                t[:, fs], x_t[(i0 + j) * 3 + c, :, k * SW:(k + 1) * SW]
                            )

                d1 = wk.tile([P, FD], F16, tag="d1")
                d3 = wk.tile([P, FD], F16, tag="d3")
                delta = wk.tile([P, FD], F16, tag="delta")
                Pt = wk.tile([P, FD], F16, tag="P")
                q = wk.tile([P, FD], F32, tag="q")
                NSS = 8
                SWS = PLANE_F // NSS
                # d3 halves on Pool (r,b land first); chunk 0 runs the whole
                # pipeline per image-half with its d1 sub emitted just before
                # that half's ops, so DVE fills the g-arrival gap with work.
                for j in range(IMGS_PER_CHUNK):
                    hs = slice(j * PLANE_F, (j + 1) * PLANE_F)
                    G.tensor_tensor(d3[:, hs], r[:, hs], b[:, hs], op=Alu.subtract)
                halves = (
                    [(j * PLANE_F, (j + 1) * PLANE_F) for j in range(IMGS_PER_CHUNK)]
                    if ch == 0 else [(0, FD)]
                )
                if ch != 0:
                    for j in range(IMGS_PER_CHUNK):
                        hs = slice(j * PLANE_F, (j + 1) * PLANE_F)
                        V.tensor_tensor(d1[:, hs], r[:, hs], g[:, hs], op=Alu.subtract)
                for lo, hi in halves:
                    h = slice(lo, hi)
                    if ch == 0:
                        V.tensor_tensor(d1[:, h], r[:, h], g[:, h], op=Alu.subtract)
                    V._custom_dve(
                        ops["HUE_DELTA_ANT"], out=delta[:, h], in0=d1[:, h],
                        in1=d3[:, h], s0=2.0 ** -15,
                    )
                    _act_reciprocal(nc, delta[:, h], delta[:, h])
                    V._custom_dve(
                        ops["HUE_PSEL_ANT"], out=Pt[:, h], in0=d1[:, h], in1=d3[:, h]
                    )
                    V._custom_dve(
                        ops["HUE_W_ANT"], out=d1[:, h], in0=d1[:, h], in1=d3[:, h]
                    )
                    V.tensor_tensor(Pt[:, h], Pt[:, h], delta[:, h], op=Alu.mult)
                    V._custom_dve(
                        ops["HUE_FIN_ANT"], out=q[:, h], in0=Pt[:, h], in1=d1[:, h],
                        s0=w6, s1=2.0 * w6, imm2=4.0 * w6 + bias,
                    )
                    for j in range(lo // PLANE_F, hi // PLANE_F):
                        for k in range(NSS):
                            fs = slice(
                                j * PLANE_F + k * SWS, j * PLANE_F + (k + 1) * SWS
                            )
                            eng = nc.scalar if (k % 2 == 0) else nc.sync
                            eng.dma_start(
                                o_t[i0 + j, :, k * SWS:(k + 1) * SWS], q[:, fs]
                            )

    nc.compile()
    return nc


def _get_nc(w6: float, bias: float):
    key = (w6, bias)
    if key not in _EXE_CACHE:
        _EXE_CACHE[key] = _build(w6, bias)
    return _EXE_CACHE[key]


def _run(x, W, b, trace=False, tmpdir=None):
    from concourse.bass_utils import run_bass_kernel_spmd

    x = np.ascontiguousarray(np.asarray(x, dtype=np.float32))
    Wv = float(np.asarray(W).reshape(-1)[0])
    bv = float(np.asarray(b).reshape(-1)[0])
    w6 = Wv / 6.0

    nc = _get_nc(w6, bv)

    shards = x.reshape(N_CORES, IMGS_PER_CORE * 3, P, PLANE_F)
    in_maps = [{"x": shards[i]} for i in range(N_CORES)]
    res = run_bass_kernel_spmd(
        nc, in_maps, list(range(N_CORES)), trace=trace, tmpdir=tmpdir
    )
    out = np.stack([res.results[i]["out"] for i in range(N_CORES)], axis=0)
    out = out.reshape(32, 1, 512, 512)
    return out, res


def kernel(x, W, b):
    out, _ = _run(x, W, b, trace=False)
    return out
